# revision 16
# baseline (speedup 1.0000x reference)
"""MoE layer (B=2,S=1024,H=2048,F=5504,E=8,top-2) on 8 NeuronCores.

Pair-split expert-parallel: experts are paired (heavy, light) to minimize
the max pair load; each pair runs on 2 cores, both processing ALL of the
pair's tokens but each over half of the 43 f-tiles (21 full tiles each,
plus the odd tile 21 split by token halves).  The host sums the two cores'
partial y, applies combine probs, and scatter-adds.

Uniform SPMD slot layout: seg1 = S1 slots (heavy expert), seg2 = S2 slots
(light expert); the partial tile processes the first half of each segment,
and core B's in_map permutes the slot axis (swap halves within segments)
so that B's "first half" is the global second half.  Down-proj partials
are emitted per f-group in bf16 and summed on the host.
"""

import sys

import numpy as np
import ml_dtypes

if "/opt/trn_rl_repo" not in sys.path:
    sys.path.insert(0, "/opt/trn_rl_repo")

B, S, H, F, E, TOPK = 2, 1024, 2048, 5504, 8, 2
T = B * S
P = 128
FT = F // P  # 43 f-tiles of 128
HC = H // P  # 16 h-chunks of 128
NT = 22  # f-tile slots per core: 21 full + 1 partial (global tile 21)
NCORES = 8
BF16 = ml_dtypes.bfloat16

_nc_cache: dict = {}

TRACE = False
LAST_RESULT = None


def _split_waits(nc):
    """Walrus encodes at most ONE sync wait per DMA-queue instruction and
    refuses multi-wait drains; split Tile's multi-wait DMAs into single-wait
    gates on the same queue (see baseline kernel for details)."""
    import copy
    import concourse.mybir as mybir

    tmpl = None
    for f in nc.m.functions:
        for b in f.blocks:
            for ins in b.instructions:
                if type(ins).__name__ == "InstDMACopy" and ins.outs and "wgdst" in str(
                    ins.outs[0]
                ):
                    tmpl = ins
    assert tmpl is not None, "wait-gate template (wgdst dma) not found"

    k = 0
    for f in nc.m.functions:
        for b in f.blocks:
            newlist = []
            for ins in b.instructions:
                si = ins.sync_info
                tn = type(ins).__name__
                if (
                    tn == "InstDMACopy"
                    and ins.name != tmpl.name
                    and si is not None
                    and si.on_wait
                    and len(si.on_wait) > 1
                ):
                    waits = list(si.on_wait)
                    for w in waits[:-1]:
                        k += 1
                        upd = copy.deepcopy(list(tmpl.sync_info.on_update))
                        for u in upd:
                            u.update_value = 0
                        d = mybir.InstDMACopy(
                            name=f"I-{900000 + k}",
                            engine=ins.engine,
                            ins=copy.deepcopy(tmpl.ins),
                            outs=copy.deepcopy(tmpl.outs),
                            queue=getattr(ins, "queue", None) or tmpl.queue,
                            mode=tmpl.mode,
                            oob_is_err=tmpl.oob_is_err,
                            cce_op=tmpl.cce_op,
                            single_packet=tmpl.single_packet,
                            sync_info=mybir.SyncInfo(on_wait=[w], on_update=upd),
                        )
                        newlist.append(d)
                    ins.sync_info = mybir.SyncInfo(
                        on_wait=[waits[-1]], on_update=list(si.on_update or [])
                    )
                elif si is not None and si.on_wait and len(si.on_wait) > 1:
                    waits = list(si.on_wait)
                    for w in waits[:-1]:
                        k += 1
                        d = mybir.InstEventSemaphore(
                            name=f"I-{900000 + k}",
                            engine=ins.engine,
                            sync_info=mybir.SyncInfo(on_wait=[w], on_update=[]),
                        )
                        newlist.append(d)
                    ins.sync_info = mybir.SyncInfo(
                        on_wait=[waits[-1]], on_update=list(si.on_update or [])
                    )
                newlist.append(ins)
            b.instructions[:] = newlist
    return k


def _build_nc(S1: int, S2: int, split: bool = True):
    import concourse.bass as bass
    import concourse.mybir as mybir
    from concourse.tile import TileContext
    from contextlib import ExitStack

    dt = mybir.dt
    assert S1 % 2 == 0 and S2 % 2 == 0
    npad = S1 + S2
    h1, h2 = S1 // 2, S2 // 2
    assert h1 <= 512 and h2 <= 512
    # (t0, n, seg); partial tile covers the first ttile of each segment
    ttiles = [(0, h1, 0), (h1, h1, 0), (S1, h2, 1), (S1 + h2, h2, 1)]
    partial_tts = (0, 2)

    # down-proj slot groups; each emits a bf16 partial y (host sums them)
    groups = [list(range(0, 8)), list(range(8, 15)), list(range(15, NT))]
    NG = len(groups)

    nc = bass.Bass()
    xt = nc.dram_tensor("xt", [P, HC, npad], dt.bfloat16, kind="ExternalInput")
    gt = nc.dram_tensor("gt", [NT, 2, P, H], dt.bfloat16, kind="ExternalInput")
    ut = nc.dram_tensor("ut", [NT, 2, P, H], dt.bfloat16, kind="ExternalInput")
    dw = nc.dram_tensor("dw", [NT, 2, P, H], dt.bfloat16, kind="ExternalInput")
    # [group, hc-quad, P, 4, npad]: 4-hc rows keep output DMA packets big
    yout = nc.dram_tensor(
        "yout", [NG, HC // 4, P, 4, npad], dt.bfloat16, kind="ExternalOutput"
    )
    wgsrc = nc.dram_tensor("wgsrc", [1, 1], dt.bfloat16, kind="ExternalInput")
    wgdst = nc.dram_tensor("wgdst", [1, 1], dt.bfloat16)

    with TileContext(nc) as tc, ExitStack() as ctx:
        cpool = ctx.enter_context(tc.tile_pool(name="const", bufs=1))
        gpool = ctx.enter_context(tc.tile_pool(name="gw", bufs=2))
        upool = ctx.enter_context(tc.tile_pool(name="uw", bufs=2))
        dpool = ctx.enter_context(tc.tile_pool(name="dwp", bufs=17))
        spool = ctx.enter_context(tc.tile_pool(name="stmp", bufs=2))
        ypool = ctx.enter_context(tc.tile_pool(name="ybp", bufs=2))
        # PSUM: pgu 2 tags x 2 bufs = 4 banks, pdn 1 tag x 4 bufs = 4 banks
        pgu = ctx.enter_context(tc.tile_pool(name="pgu", bufs=2, space="PSUM"))
        pdn = ctx.enter_context(tc.tile_pool(name="pdn", bufs=4, space="PSUM"))

        x_sb = cpool.tile([P, HC, npad], dt.bfloat16, tag="x")
        for c in range(0, HC, 4):
            nc.gpsimd.dma_start(x_sb[:, c : c + 4, :], xt[:, c : c + 4, :])
        h_sb = cpool.tile([P, NT, npad], dt.bfloat16, tag="h")

        # dw tiles per (slot, seg) live in dpool; loaded by the group emitter
        def emit_down_group(gi: int):
            grp = groups[gi]
            dts = {}
            for ts in grp:
                for seg in (0, 1):
                    dtile = dpool.tile([P, H], dt.bfloat16, tag="dw", name="dtile")
                    if gi == 0:
                        # artificial dep on the x stream: the per-core DMA
                        # fabric is shared across queues, so an eager dw
                        # prefetch starves the startup-critical x load
                        nc.vector.tensor_copy(
                            dtile[0:1, 0:1], x_sb[0:1, HC - 1, npad - 1 : npad]
                        )
                    nc.scalar.dma_start(dtile[:], dw[ts, seg])
                    dts[(ts, seg)] = dtile
            ybt = None
            for hc in range(HC):
                if hc % 4 == 0:
                    ybt = ypool.tile(
                        [P, 4, npad], dt.bfloat16, tag="yb", name="ybt"
                    )
                for tti, (t0, n, seg) in enumerate(ttiles):
                    slots = [
                        ts
                        for ts in grp
                        if ts != NT - 1 or tti in partial_tts
                    ]
                    ps = pdn.tile([P, 512], dt.float32, tag="dn", name="ps")
                    for j, ts in enumerate(slots):
                        nc.tensor.matmul(
                            ps[:, :n],
                            dts[(ts, seg)][:, hc * P : (hc + 1) * P],
                            h_sb[:, ts, t0 : t0 + n],
                            start=(j == 0),
                            stop=(j == len(slots) - 1),
                        )
                    nc.vector.tensor_copy(
                        ybt[:, hc % 4, t0 : t0 + n], ps[:, :n]
                    )
                if hc % 4 == 3:
                    nc.gpsimd.dma_start(yout[gi, hc // 4], ybt[:])

        next_grp = 0
        for ts in range(NT):
            partial = ts == NT - 1
            g1 = gpool.tile([P, H], dt.bfloat16, tag="g1", name="g1")
            nc.sync.dma_start(g1[:], gt[ts, 0])
            g2 = gpool.tile([P, H], dt.bfloat16, tag="g2", name="g2")
            nc.sync.dma_start(g2[:], gt[ts, 1])
            u1 = upool.tile([P, H], dt.bfloat16, tag="u1", name="u1")
            nc.sync.dma_start(u1[:], ut[ts, 0])
            u2 = upool.tile([P, H], dt.bfloat16, tag="u2", name="u2")
            nc.sync.dma_start(u2[:], ut[ts, 1])
            gsl = (g1, g1, g2, g2)
            usl = (u1, u1, u2, u2)

            # per segment: gate psums, up psums, then silu/mul (pgu bufs=2)
            for seg_tts in ((0, 1), (2, 3)):
                live = [
                    tti for tti in seg_tts if not partial or tti in partial_tts
                ]
                psg, psu = {}, {}
                for tti in live:
                    t0, n, seg = ttiles[tti]
                    gp = pgu.tile([P, 512], dt.float32, tag="gp", name="gp")
                    for hc in range(HC):
                        nc.tensor.matmul(
                            gp[:, :n],
                            gsl[tti][:, hc * P : (hc + 1) * P],
                            x_sb[:, hc, t0 : t0 + n],
                            start=(hc == 0),
                            stop=(hc == HC - 1),
                        )
                    psg[tti] = gp
                for tti in live:
                    t0, n, seg = ttiles[tti]
                    up = pgu.tile([P, 512], dt.float32, tag="up", name="up")
                    for hc in range(HC):
                        nc.tensor.matmul(
                            up[:, :n],
                            usl[tti][:, hc * P : (hc + 1) * P],
                            x_sb[:, hc, t0 : t0 + n],
                            start=(hc == 0),
                            stop=(hc == HC - 1),
                        )
                    psu[tti] = up
                for tti in live:
                    t0, n, seg = ttiles[tti]
                    st = spool.tile([P, 512], dt.float32, tag="st", name="st")
                    nc.scalar.activation(
                        st[:, :n],
                        psg[tti][:, :n],
                        mybir.ActivationFunctionType.Sigmoid,
                    )
                    nc.vector.tensor_mul(
                        out=st[:, :n], in0=st[:, :n], in1=psg[tti][:, :n]
                    )
                    nc.vector.tensor_mul(
                        out=h_sb[:, ts, t0 : t0 + n],
                        in0=st[:, :n],
                        in1=psu[tti][:, :n],
                    )

            if next_grp < len(groups) and ts == groups[next_grp][-1]:
                emit_down_group(next_grp)
                next_grp += 1

        nc.sync.dma_start(wgdst[:], wgsrc[:])

    if split:
        _split_waits(nc)
    return nc


def _route(xf: np.ndarray, router_w: np.ndarray):
    logits = xf.astype(np.float64) @ router_w.astype(np.float64).T  # [T, E]
    order = np.argsort(-logits, axis=-1, kind="stable")[:, :TOPK]  # [T, 2]
    top_v = np.take_along_axis(logits, order, axis=1)
    ex = np.exp(top_v - top_v.max(axis=1, keepdims=True))
    probs = (ex / ex.sum(axis=1, keepdims=True)).astype(np.float32)
    return order, probs


def _perm_B(S1: int, S2: int) -> np.ndarray:
    """Core-B slot permutation: local slot j holds global slot perm[j]
    (swap halves within each segment)."""
    p = np.empty(S1 + S2, dtype=np.int64)
    h1, h2 = S1 // 2, S2 // 2
    p[:h1] = np.arange(h1, S1)
    p[h1:S1] = np.arange(0, h1)
    p[S1 : S1 + h2] = S1 + np.arange(h2, S2)
    p[S1 + h2 :] = S1 + np.arange(0, h2)
    return p


def _prep_w(gate_w, up_w, down_w, e: int):
    """Per-expert weight tiles: g/u as [FT, P(h), H->(hc,fi)] lhsT layout,
    dw as [FT, P(f), H]."""
    g16 = gate_w[e].astype(BF16)
    u16 = up_w[e].astype(BF16)
    d16 = down_w[e].astype(BF16)
    gtt = np.ascontiguousarray(
        g16.reshape(FT, P, HC, P).transpose(0, 3, 2, 1)
    ).reshape(FT, P, HC * P)
    utt = np.ascontiguousarray(
        u16.reshape(FT, P, HC, P).transpose(0, 3, 2, 1)
    ).reshape(FT, P, HC * P)
    dtt = np.ascontiguousarray(d16.T).reshape(FT, P, H)
    return gtt, utt, dtt


def kernel(x, router_w, gate_w, up_w, down_w):
    from concourse.bass_utils import run_bass_kernel_spmd

    x = np.asarray(x)
    router_w = np.asarray(router_w)
    gate_w = np.asarray(gate_w)
    up_w = np.asarray(up_w)
    down_w = np.asarray(down_w)

    xf = x.reshape(T, H)
    order, probs = _route(xf, router_w)

    idxs, pes = [], []
    for e in range(E):
        sel = (order[:, 0] == e) | (order[:, 1] == e)
        idx = np.nonzero(sel)[0]
        pe = np.where(order[idx, 0] == e, probs[idx, 0], probs[idx, 1])
        idxs.append(idx)
        pes.append(pe.astype(np.float32))

    counts = np.array([len(i) for i in idxs])
    srt = np.argsort(-counts)
    pairs = [(int(srt[i]), int(srt[7 - i])) for i in range(4)]
    S1 = -(-max(counts[a] for a, _ in pairs) // 2) * 2
    S2 = -(-max(counts[b] for _, b in pairs) // 2) * 2
    S1, S2 = max(S1, 2 * P), max(S2, 2 * P)
    npad = S1 + S2
    perm = _perm_B(S1, S2)

    if (S1, S2) not in _nc_cache:
        _nc_cache[(S1, S2)] = _build_nc(S1, S2)
    nc = _nc_cache[(S1, S2)]

    # per-core f-tile slot lists: A = [0..20] + [21], B = [22..42] + [21]
    tiles_A = list(range(21)) + [21]
    tiles_B = list(range(22, FT)) + [21]

    in_maps = []
    wcache = {}
    for ea, eb in pairs:
        if ea not in wcache:
            wcache[ea] = _prep_w(gate_w, up_w, down_w, ea)
        if eb not in wcache:
            wcache[eb] = _prep_w(gate_w, up_w, down_w, eb)
        ga, ua, da = wcache[ea]
        gb, ub, db = wcache[eb]

        xs = np.zeros((npad, H), dtype=BF16)
        xs[: counts[ea]] = xf[idxs[ea]].astype(BF16)
        xs[S1 : S1 + counts[eb]] = xf[idxs[eb]].astype(BF16)

        for core, tiles in ((0, tiles_A), (1, tiles_B)):
            xl = xs if core == 0 else xs[perm]
            xtt = np.ascontiguousarray(xl.reshape(npad, HC, P).transpose(2, 1, 0))
            gtt = np.stack([ga[tiles], gb[tiles]], axis=1)  # [NT, 2, P, H]
            utt = np.stack([ua[tiles], ub[tiles]], axis=1)
            dtt = np.stack([da[tiles], db[tiles]], axis=1)
            in_maps.append(
                {
                    "xt": xtt,
                    "gt": np.ascontiguousarray(gtt),
                    "ut": np.ascontiguousarray(utt),
                    "dw": np.ascontiguousarray(dtt),
                    "wgsrc": np.zeros((1, 1), dtype=BF16),
                }
            )

    res = run_bass_kernel_spmd(
        nc, in_maps, core_ids=list(range(NCORES)), trace=TRACE
    )
    global LAST_RESULT
    LAST_RESULT = res

    out_flat = np.zeros((T, H), dtype=np.float32)
    for pi, (ea, eb) in enumerate(pairs):
        # yout[g, c, p, j, t] = partial_y[(4c+j)*128+p, t]
        def unpack(r):
            return (
                r["yout"]
                .astype(np.float32)
                .sum(axis=0)
                .transpose(0, 2, 1, 3)
                .reshape(H, npad)
            )

        yA = unpack(res.results[2 * pi])
        yBl = unpack(res.results[2 * pi + 1])
        yB = np.zeros_like(yBl)
        yB[:, perm] = yBl
        y = yA + yB
        out_flat[idxs[ea]] += y[:, : counts[ea]].T * pes[ea][:, None]
        out_flat[idxs[eb]] += y[:, S1 : S1 + counts[eb]].T * pes[eb][:, None]
    return out_flat.reshape(B, S, H)
